# revision 1
# baseline (speedup 1.0000x reference)
"""BiMamba (bidirectional Mamba block + LN + FFN) Trainium2 Bass kernel.

Sharding (8 cores): 4 scan-sequences (fwd/bwd x batch, bwd fed host-flipped x)
x 2 halves of d_inner. Device layout is feature-on-partitions /
time-on-free throughout; the host transposes x on the way in and the output
on the way out. Cross-core combines (out_proj partial sums + direction
merge, ff2 partial sums) use AllGather/AllReduce over quads
[0,1,4,5] / [2,3,6,7].
"""
import sys, os, types, contextlib, ctypes

sys.path.insert(0, "/opt/trn_rl_repo")
import numpy as np

D_MODEL = 1024
D_STATE = 16
D_CONV = 4
D_INNER = 2048
DT_RANK = 64
L = 1024
HALF = D_INNER // 2          # 1024 d_inner per core
P = 128
NJ = HALF // P               # 8 d-blocks per core half
TCH = 512                    # matmul t-chunk
NT = L // TCH
KD = D_MODEL // P            # 8 k-chunks over d_model
NFB_XC = D_INNER // P        # 16 xc feature blocks (full d_inner)
FF_SLICE = 1024              # ffn hidden slice per core
NB = DT_RANK + 2 * D_STATE   # 96

_GROUPS = [[0, 1, 4, 5], [2, 3, 6, 7]]


def _install_ntff_hook_shim(so_path="/opt/axon/libaxon_pjrt.so"):
    if "antenv.axon_hooks" in sys.modules:
        return
    try:
        lib = ctypes.CDLL(so_path)
    except OSError:
        return
    if not hasattr(lib, "axon_start_nrt_profile"):
        return
    lib.axon_start_nrt_profile.argtypes = [ctypes.POINTER(ctypes.c_int64), ctypes.c_size_t]
    lib.axon_start_nrt_profile.restype = ctypes.c_int64
    lib.axon_stop_nrt_profile.argtypes = [ctypes.c_char_p]
    lib.axon_stop_nrt_profile.restype = ctypes.c_int64

    @contextlib.contextmanager
    def _hook(output_dir, device_ids):
        import jax
        jax.devices()
        if device_ids:
            ids = (ctypes.c_int64 * len(device_ids))(*device_ids)
            rc = lib.axon_start_nrt_profile(ids, len(device_ids))
        else:
            rc = lib.axon_start_nrt_profile(None, 0)
        if rc != 0:
            raise RuntimeError(f"axon_start_nrt_profile rc={rc}")
        try:
            yield
        finally:
            n = lib.axon_stop_nrt_profile(str(output_dir).encode())
            print(f"profile: {n} file(s) written to {output_dir}", file=sys.stderr)

    mod = types.ModuleType("antenv.axon_hooks")
    mod.get_axon_ntff_profile_hook = lambda: _hook
    mod.set_axon_ntff_profile_hook = lambda h: None
    sys.modules["antenv.axon_hooks"] = mod


def _build_nc():
    from concourse import bacc, tile, mybir

    f32 = mybir.dt.float32
    f32r = mybir.dt.float32r
    bf16 = mybir.dt.bfloat16
    Alu = mybir.AluOpType
    Act = mybir.ActivationFunctionType

    def r(ap):
        return ap.bitcast(f32r)

    nc = bacc.Bacc("TRN2", target_bir_lowering=False, debug=False, num_devices=8)

    def din(name, shape, dt=None):
        return nc.dram_tensor(name, list(shape), dt or f32, kind="ExternalInput").ap()

    xT = din("xT", (D_MODEL, L), f32r)
    w_in_t = din("w_in_t", (NJ + NFB_XC, KD, P, P), f32r)          # z-half blocks, then xc
    convw_cols = din("convw_cols", (P, NFB_XC * D_CONV))
    convb_cols = din("convb_cols", (P, NFB_XC))
    xpw_t = din("xpw_t", (NFB_XC, P, P), f32r)  # cols: dt64|B16|pad|C16|pad
    dtw_t = din("dtw_t", (NJ, DT_RANK, P), f32r)
    dtb_cols = din("dtb_cols", (P, NJ))
    A_cols = din("A_cols", (P, NJ * D_STATE))
    D_colsT = din("D_colsT", (P, NJ))
    outw_t = din("outw_t", (KD, NJ, P, P), f32r)                   # [k(d_in), m(dm)]
    lng_cols = din("lng_cols", (P, KD))
    lnb_cols = din("lnb_cols", (P, KD))
    w1_t = din("w1_t", (KD, NJ, P, P), f32r)                       # [k(dm), m(h)]
    b1_cols = din("b1_cols", (P, NJ))
    w2_t = din("w2_t", (NJ, KD, P, P), f32r)                       # [k(h), m(dm)]
    b2_cols = din("b2_cols", (P, 2))
    consts_r = din("consts_r", (P, 4), f32r)  # col0=1/1024, cols1..3=0
    ident_r = din("ident_r", (P, P), f32r)

    out_m = nc.dram_tensor("out_m", [D_MODEL // 4, L], f32, kind="ExternalOutput").ap()

    es = contextlib.ExitStack()

    with tile.TileContext(nc) as tc:
        with contextlib.ExitStack() as stk:
            cpool = stk.enter_context(tc.tile_pool(name="cpool", bufs=1))
            psum = stk.enter_context(tc.tile_pool(name="psum", bufs=4, space="PSUM"))
            dram = stk.enter_context(tc.tile_pool(name="dram", bufs=1, space="DRAM"))

            def cload(src, shape, tag):
                t = cpool.tile(list(shape), f32, tag=tag, name=tag)
                nc.sync.dma_start(t[:], src)
                return t

            A_sb = cload(A_cols[:], (P, NJ * D_STATE), "A_sb")
            dtb_sb = cload(dtb_cols[:], (P, NJ), "dtb_sb")
            D_sb = cload(D_colsT[:], (P, NJ), "D_sb")
            convb_sb = cload(convb_cols[:], (P, NFB_XC), "convb_sb")
            convw_sb = cload(convw_cols[:], (P, NFB_XC * D_CONV), "convw_sb")
            lng_sb = cload(lng_cols[:], (P, KD), "lng_sb")
            lnb_sb = cload(lnb_cols[:], (P, KD), "lnb_sb")
            b1_sb = cload(b1_cols[:], (P, NJ), "b1_sb")
            b2_sb = cload(b2_cols[:], (P, 2), "b2_sb")
            ones_sb = cpool.tile([P, 1], f32r, tag="ones_sb", name="ones_sb")
            nc.sync.dma_start(ones_sb[:], consts_r[:, 0:1])
            ident_sb = cpool.tile([P, P], f32r, tag="ident_sb", name="ident_sb")
            nc.sync.dma_start(ident_sb[:], ident_r[:])

            bcB_dram = dram.tile([D_STATE, L], bf16, name="bcB_dram")
            bcC_dram = dram.tile([D_STATE, L], f32, name="bcC_dram")
            stat_dram = dram.tile([2, L], f32, name="stat_dram")
            HD = D_MODEL // 2
            ag_in = [dram.tile([HD, L], f32, name=f"ag_in{h}") for h in range(2)]
            ag_out = [dram.tile([4 * HD, L], f32, name=f"ag_out{h}") for h in range(2)]
            ar_in = dram.tile([D_MODEL, L], f32, name="ar_in")
            rs_out = dram.tile([D_MODEL // 4, L], f32, name="rs_out")

            def mm_accum(ps, lw_list, rhs_of_k, n_k):
                for k in range(n_k):
                    nc.tensor.matmul(ps[:], lw_list[k][:], rhs_of_k(k),
                                     start=(k == 0), stop=(k == n_k - 1))

            # =========== P1-P4 region: sz lives until gating ===========
            with tc.tile_pool(name="sz_pool", bufs=1) as sz_pool:
                sz = [sz_pool.tile([P, L], f32, tag=f"sz{j}", name=f"sz{j}")
                      for j in range(NJ)]
                dt_sb = sz_pool.tile([DT_RANK, L], f32r, tag="dt_sb", name="dt_sb")

                def dt_proj_delta(j, delta_t, pool):
                    # softplus(x + b) = Ln(1 + Exp(x + b)); inputs here are
                    # well below 0 so Exp cannot overflow.
                    lw = pool.tile([DT_RANK, P], f32r, tag="dtw", name=f"dtw{j}", bufs=2)
                    nc.sync.dma_start(lw[:], dtw_t[j])
                    for t in range(NT):
                        ps = psum.tile([P, TCH], f32, tag="ps", name=f"dtp{j}_{t}")
                        nc.tensor.matmul(ps[:], lw[:],
                                         dt_sb[:, t * TCH:(t + 1) * TCH],
                                         start=True, stop=True)
                        spt = pool.tile([P, TCH], f32, tag="spt", name=f"spt{j}_{t}",
                                        bufs=2)
                        nc.scalar.activation(spt[:], ps[:], Act.Exp,
                                             bias=dtb_sb[:, j:j + 1])
                        nc.scalar.activation(delta_t[:, t * TCH:(t + 1) * TCH], spt[:],
                                             Act.Ln, bias=1.0)

                # ---------------- P1..P3: need xc blocks ----------------
                with tc.tile_pool(name="xc_pool", bufs=1) as xc_pool:
                    xcs = [xc_pool.tile([P, L], f32r, tag=f"xcs{j}", name=f"xcs{j}")
                           for j in range(NFB_XC)]

                    # P1: in_proj + conv + silu
                    with tc.tile_pool(name="xt_pool", bufs=1) as xt_pool, \
                         tc.tile_pool(name="p1t", bufs=1) as p1t:
                        xts = []
                        for k in range(KD):
                            xt_k = xt_pool.tile([P, L], f32r, tag=f"xt{k}", name=f"xt{k}")
                            nc.sync.dma_start(xt_k[:], xT[k * P:(k + 1) * P, :])
                            xts.append(xt_k)

                        def in_proj_block(fb):
                            lws = []
                            for k in range(KD):
                                lw = p1t.tile([P, P], f32r, tag=f"lw{k}",
                                              name=f"lw{fb}_{k}", bufs=2)
                                nc.sync.dma_start(lw[:], w_in_t[fb, k])
                                lws.append(lw)
                            pss = []
                            for t in range(NT):
                                ps = psum.tile([P, TCH], f32, tag="ps",
                                               name=f"inp{fb}_{t}")
                                mm_accum(ps, lws,
                                         lambda k: xts[k][:, t * TCH:(t + 1) * TCH], KD)
                                pss.append(ps)
                            return pss

                        for j in range(NJ):  # z half
                            for t, ps in enumerate(in_proj_block(j)):
                                nc.scalar.activation(sz[j][:, t * TCH:(t + 1) * TCH],
                                                     ps[:], Act.Silu)

                        for j in range(NFB_XC):  # xc blocks + conv (DVE taps)
                            xcp = p1t.tile([P, L + D_CONV - 1], f32, tag="xcp",
                                           name=f"xcp{j}", bufs=2)
                            nc.sync.dma_start(xcp[:, 0:D_CONV - 1],
                                              consts_r[:, 1:D_CONV].bitcast(f32))
                            for t, ps in enumerate(in_proj_block(NJ + j)):
                                nc.scalar.copy(
                                    xcp[:, D_CONV - 1 + t * TCH:D_CONV - 1 + (t + 1) * TCH],
                                    ps[:])
                            cacc = p1t.tile([P, L], f32, tag="cacc",
                                            name=f"cacc{j}", bufs=2)
                            nc.vector.tensor_scalar_mul(
                                cacc[:], xcp[:, 0:L],
                                convw_sb[:, j * D_CONV:j * D_CONV + 1])
                            for i in range(1, D_CONV):
                                nc.vector.scalar_tensor_tensor(
                                    cacc[:], xcp[:, i:i + L],
                                    convw_sb[:, j * D_CONV + i:j * D_CONV + i + 1],
                                    cacc[:], Alu.mult, Alu.add)
                            nc.scalar.activation(xcs[j][:], cacc[:], Act.Silu,
                                                 bias=convb_sb[:, j:j + 1])

                    # right-side pools for wv/g0 (live P2..P4)
                    wvg_ctx = contextlib.ExitStack()
                    wv_pool = wvg_ctx.enter_context(
                        tc.tile_pool(name="wv_pool", bufs=1, side="right"))
                    wvs = [wv_pool.tile([P, L], bf16, tag=f"wv{j}", name=f"wv{j}")
                           for j in range(NJ)]
                    g0_pool = wvg_ctx.enter_context(
                        tc.tile_pool(name="g0_pool", bufs=1, side="right"))
                    g0s = [g0_pool.tile([P, L], f32, tag=f"g0{j}", name=f"g0{j}")
                           for j in range(NJ)]

                    # P2: x_proj; P3: wv/g0
                    with tc.tile_pool(name="p2t", bufs=1) as p2t:
                        for t in range(NT):
                            ps = psum.tile([P, TCH], f32, tag="ps", name=f"xproj{t}")
                            for k in range(NFB_XC):
                                lw = p2t.tile([P, P], f32r, tag="xpw",
                                              name=f"xpw{t}_{k}", bufs=2)
                                nc.sync.dma_start(lw[:], xpw_t[k])
                                nc.tensor.matmul(ps[:], lw[:],
                                                 xcs[k][:, t * TCH:(t + 1) * TCH],
                                                 start=(k == 0), stop=(k == NFB_XC - 1))
                            nc.scalar.copy(dt_sb[:, t * TCH:(t + 1) * TCH],
                                           ps[0:DT_RANK, :])
                            bcB_sb = p2t.tile([D_STATE, TCH], bf16, tag="bcB_sb",
                                              name=f"bcB_sb{t}", bufs=2)
                            nc.scalar.copy(bcB_sb[:], ps[64:80, :])
                            nc.sync.dma_start(bcB_dram[:, t * TCH:(t + 1) * TCH], bcB_sb[:])
                            bcC_sb = p2t.tile([D_STATE, TCH], f32, tag="bcC_sb",
                                              name=f"bcC_sb{t}", bufs=2)
                            nc.scalar.copy(bcC_sb[:], ps[96:112, :])
                            nc.sync.dma_start(bcC_dram[:, t * TCH:(t + 1) * TCH], bcC_sb[:])

                        for j in range(NJ):
                            dtmp = p2t.tile([P, L], f32, tag="dtmp", name=f"dtmp{j}",
                                            bufs=2)
                            dt_proj_delta(j, dtmp, p2t)
                            nc.vector.tensor_tensor(wvs[j][:], dtmp[:], xcs[j][:],
                                                    Alu.mult)
                            t1 = p2t.tile([P, L], f32, tag="g0tmp", name=f"g0tmp{j}",
                                          bufs=2)
                            nc.vector.tensor_scalar_mul(t1[:], xcs[j][:], D_sb[:, j:j + 1])
                            nc.vector.tensor_tensor(g0s[j][:], t1[:], sz[j][:], Alu.mult)

                # ---------------- P4: scan + gating ----------------
                yg_ctx = contextlib.ExitStack()
                yg_pool = yg_ctx.enter_context(tc.tile_pool(name="yg_pool", bufs=1))
                ygs = [yg_pool.tile([P, L], f32r, tag=f"yg{j}", name=f"yg{j}")
                       for j in range(NJ)]
                with tc.tile_pool(name="spool", bufs=1) as spool, \
                     tc.tile_pool(name="tpool", bufs=1) as tpool, \
                     tc.tile_pool(name="pscan", bufs=1, space="PSUM") as pscan:
                    for hb in range(4):
                        js = list(range(hb * 2, hb * 2 + 2))
                        deltas = {}
                        yps = {}
                        for j in js:
                            dj = spool.tile([P, L], f32, tag=f"delta{j % 2}",
                                            name=f"delta{j}")
                            dt_proj_delta(j, dj, spool)
                            deltas[j] = dj
                            yps[j] = pscan.tile([P, L], f32, tag=f"yps{j % 2}",
                                                name=f"yps{j}")
                        for n in range(D_STATE):
                            Bbc = tpool.tile([P, L], bf16, tag="Bbc",
                                             name=f"Bbc{hb}_{n}", bufs=2)
                            nc.sync.dma_start(
                                Bbc[:],
                                bcB_dram[n:n + 1, :].partition_broadcast(P).squeeze(1))
                            Cbc = tpool.tile([P, L], f32, tag="Cbc",
                                             name=f"Cbc{hb}_{n}", bufs=2)
                            nc.sync.dma_start(
                                Cbc[:],
                                bcC_dram[n:n + 1, :].partition_broadcast(P).squeeze(1))
                            for j in js:
                                a_t = tpool.tile([P, L], f32, tag="a_t",
                                                 name=f"a{j}_{n}", bufs=3)
                                nc.scalar.activation(
                                    a_t[:], deltas[j][:], Act.Exp,
                                    scale=A_sb[:, j * D_STATE + n:j * D_STATE + n + 1])
                                b_t = tpool.tile([P, L], bf16, tag="b_t",
                                                 name=f"b{j}_{n}", bufs=1)
                                nc.vector.tensor_tensor(b_t[:], wvs[j][:], Bbc[:],
                                                        Alu.mult)
                                h_t = tpool.tile([P, L], f32, tag="h_t",
                                                 name=f"h{j}_{n}", bufs=2)
                                nc.vector.tensor_tensor_scan(h_t[:], a_t[:], b_t[:],
                                                             0.0, Alu.mult, Alu.add)
                                prod = tpool.tile([P, L], f32r, tag="prod",
                                                  name=f"p{j}_{n}", bufs=3)
                                nc.vector.tensor_tensor(prod[:], h_t[:], Cbc[:],
                                                        Alu.mult)
                                for t in range(NT):
                                    sl = slice(t * TCH, (t + 1) * TCH)
                                    nc.tensor.matmul(yps[j][:, sl], ident_sb[:],
                                                     prod[:, sl],
                                                     start=(n == 0),
                                                     stop=(n == D_STATE - 1))
                        for j in js:
                            ygt = tpool.tile([P, L], f32, tag="ygt",
                                             name=f"ygt{j}", bufs=2)
                            nc.vector.tensor_tensor(ygt[:], yps[j][:], sz[j][:],
                                                    Alu.mult)
                            nc.vector.tensor_tensor(ygs[j][:], ygt[:], g0s[j][:],
                                                    Alu.add)
                wvg_ctx.close()  # wv + g0 (right side, LIFO: g0 then wv)

                # =========== P5: out_proj partials + split AllGather ===========
                with tc.tile_pool(name="p5t", bufs=1) as p5t:
                    for m in range(NJ):
                        lws = []
                        for k in range(KD):
                            lw = p5t.tile([P, P], f32r, tag=f"lw{k}",
                                          name=f"ow{m}_{k}", bufs=2)
                            nc.sync.dma_start(lw[:], outw_t[k, m])
                            lws.append(lw)
                        msb = p5t.tile([P, L], f32, tag="msb", name=f"msb{m}", bufs=2)
                        for t in range(NT):
                            ps = psum.tile([P, TCH], f32, tag="ps", name=f"op{m}_{t}")
                            mm_accum(ps, lws,
                                     lambda k: ygs[k][:, t * TCH:(t + 1) * TCH], KD)
                            nc.scalar.copy(msb[:, t * TCH:(t + 1) * TCH], ps[:])
                        h = m // 4
                        nc.sync.dma_start(
                            ag_in[h][(m % 4) * P:(m % 4 + 1) * P, :], msb[:])
                        if m == 3 or m == NJ - 1:
                            nc.gpsimd.collective_compute(
                                "AllGather", Alu.bypass, replica_groups=_GROUPS,
                                ins=[ag_in[h][:]], outs=[ag_out[h][:]])
                yg_ctx.close()

            # =========== P6: mo + LN ===========
            with contextlib.ExitStack() as stk2:
                mo_pool = stk2.enter_context(tc.tile_pool(name="mo_pool", bufs=1))
                mos = [mo_pool.tile([P, L], f32r, tag=f"mo{j}", name=f"mo{j}")
                       for j in range(KD)]
                xn_pool = stk2.enter_context(tc.tile_pool(name="xn_pool", bufs=1))
                xns = [xn_pool.tile([P, L], f32r, tag=f"xn{j}", name=f"xn{j}")
                       for j in range(KD)]

                with tc.tile_pool(name="p6t", bufs=1) as p6t, \
                     tc.tile_pool(name="pln", bufs=1, space="PSUM") as pln:
                    mu_ps = pln.tile([1, L], f32, tag="mu_ps", name="mu_ps", bufs=1)
                    e2_ps = pln.tile([1, L], f32, tag="e2_ps", name="e2_ps", bufs=1)
                    for j in range(KD):
                        parts = []
                        h, jm = j // 4, j % 4
                        for q in range(4):
                            pt = p6t.tile([P, L], f32, tag="agp", name=f"agp{j}_{q}",
                                          bufs=4)
                            nc.sync.dma_start(
                                pt[:],
                                ag_out[h][q * 4 * P + jm * P:q * 4 * P + (jm + 1) * P, :])
                            parts.append(pt)
                        a01 = p6t.tile([P, L], f32, tag="a01", name=f"a01_{j}", bufs=2)
                        nc.vector.tensor_tensor(a01[:], parts[0][:], parts[1][:],
                                                Alu.add)
                        a23 = p6t.tile([P, L], f32, tag="a23", name=f"a23_{j}", bufs=2)
                        nc.vector.tensor_tensor(a23[:], parts[2][:], parts[3][:],
                                                Alu.add)
                        nc.vector.tensor_tensor(mos[j][:], a01[:], a23[:, ::-1],
                                                Alu.add)
                        sq = p6t.tile([P, L], f32r, tag="sq", name=f"sq{j}", bufs=2)
                        nc.scalar.activation(sq[:], mos[j][:], Act.Square)
                        for t in range(NT):
                            sl = slice(t * TCH, (t + 1) * TCH)
                            nc.tensor.matmul(mu_ps[:, sl], ones_sb[:],
                                             mos[j][:, sl],
                                             start=(j == 0), stop=(j == KD - 1))
                            nc.tensor.matmul(e2_ps[:, sl], ones_sb[:],
                                             sq[:, sl],
                                             start=(j == 0), stop=(j == KD - 1))

                    mean_sb = p6t.tile([1, L], f32r, tag="mean_sb", name="mean_sb",
                                       bufs=1)
                    nc.scalar.copy(mean_sb[:], mu_ps[:])
                    m2 = p6t.tile([1, L], f32, tag="m2", name="m2", bufs=1)
                    nc.vector.tensor_tensor(m2[:], mean_sb[:], mean_sb[:], Alu.mult)
                    var_t = p6t.tile([1, L], f32, tag="var_t", name="var_t", bufs=1)
                    nc.vector.tensor_tensor(var_t[:], e2_ps[:], m2[:], Alu.subtract)
                    eps_sb = p6t.tile([1, 1], f32, tag="eps_sb", name="eps_sb", bufs=1)
                    nc.vector.memset(eps_sb[:], 1e-5)
                    std_t = p6t.tile([1, L], f32, tag="std_t", name="std_t", bufs=1)
                    nc.scalar.activation(std_t[:], var_t[:], Act.Sqrt, bias=eps_sb[:])
                    rstd_sb = p6t.tile([1, L], f32r, tag="rstd_sb", name="rstd_sb",
                                       bufs=1)
                    with nc.allow_low_precision(reason="f32r view of fp32 rstd"):
                        nc.vector.reciprocal(rstd_sb[:], std_t[:])
                    onesrow = p6t.tile([1, P], f32r, tag="onesrow", name="onesrow",
                                       bufs=1)
                    nc.scalar.activation(onesrow[:], ident_sb[0:1, :], Act.Copy,
                                         bias=1.0, scale=0.0)
                    mean_bc = pln.tile([P, L], f32, tag="mu_ps", name="mean_bc",
                                       bufs=1)
                    rstd_bc = pln.tile([P, L], f32, tag="e2_ps", name="rstd_bc",
                                       bufs=1)
                    for t in range(NT):
                        sl = slice(t * TCH, (t + 1) * TCH)
                        nc.tensor.matmul(mean_bc[:, sl], onesrow[:], mean_sb[:, sl],
                                         start=True, stop=True)
                        nc.tensor.matmul(rstd_bc[:, sl], onesrow[:], rstd_sb[:, sl],
                                         start=True, stop=True)

                    for j in range(KD):
                        t1 = p6t.tile([P, L], f32, tag="lnt", name=f"lnt{j}", bufs=2)
                        nc.vector.tensor_tensor(t1[:], mos[j][:], mean_bc[:],
                                                Alu.subtract)
                        nc.vector.tensor_tensor(t1[:], t1[:], rstd_bc[:], Alu.mult)
                        nc.vector.tensor_scalar(xns[j][:], t1[:], lng_sb[:, j:j + 1],
                                                lnb_sb[:, j:j + 1], Alu.mult, Alu.add)

                # =========== P7: FFN ===========
                with tc.tile_pool(name="ffh_pool", bufs=1) as ffh_pool, \
                     tc.tile_pool(name="p7t", bufs=1) as p7t:
                    ffhs = [ffh_pool.tile([P, L], f32r, tag=f"ffh{m}", name=f"ffh{m}")
                            for m in range(NJ)]
                    for m in range(NJ):
                        lws = []
                        for k in range(KD):
                            lw = p7t.tile([P, P], f32r, tag=f"lw{k}", name=f"w1_{m}_{k}",
                                          bufs=2)
                            nc.sync.dma_start(lw[:], w1_t[k, m])
                            lws.append(lw)
                        for t in range(NT):
                            ps = psum.tile([P, TCH], f32, tag="ps", name=f"f1{m}_{t}")
                            mm_accum(ps, lws,
                                     lambda k: xns[k][:, t * TCH:(t + 1) * TCH], KD)
                            nc.scalar.activation(ffhs[m][:, t * TCH:(t + 1) * TCH],
                                                 ps[:], Act.Gelu,
                                                 bias=b1_sb[:, m:m + 1])

                    for m in range(KD):
                        lws = []
                        for k in range(NJ):
                            lw = p7t.tile([P, P], f32r, tag=f"lw{k}", name=f"w2_{m}_{k}",
                                          bufs=2)
                            nc.sync.dma_start(lw[:], w2_t[k, m])
                            lws.append(lw)
                        msb = p7t.tile([P, L], f32, tag="msb", name=f"f2sb{m}", bufs=2)
                        for t in range(NT):
                            ps = psum.tile([P, TCH], f32, tag="ps", name=f"f2{m}_{t}")
                            mm_accum(ps, lws,
                                     lambda k: ffhs[k][:, t * TCH:(t + 1) * TCH], NJ)
                            nc.scalar.copy(msb[:, t * TCH:(t + 1) * TCH], ps[:])
                        nc.sync.dma_start(ar_in[m * P:(m + 1) * P, :], msb[:])

            nc.gpsimd.collective_compute("ReduceScatter", Alu.add,
                                         replica_groups=_GROUPS,
                                         ins=[ar_in[:]], outs=[rs_out[:]])

            with tc.tile_pool(name="p8t", bufs=1) as p8t:
                for j in range(2):
                    fin = p8t.tile([P, L], f32, tag="fin", name=f"fin{j}", bufs=2)
                    nc.sync.dma_start(fin[:], rs_out[j * P:(j + 1) * P, :])
                    fob = p8t.tile([P, L], f32, tag="fob", name=f"fob{j}", bufs=2)
                    nc.vector.tensor_scalar_add(fob[:], fin[:], b2_sb[:, j:j + 1])
                    nc.sync.dma_start(out_m[j * P:(j + 1) * P, :], fob[:])

    nc.compile()
    return nc


def _prep_inputs(inputs):
    """Per-core input dicts. Core c: sequence s=c//2 (s>=2 => time-flipped x),
    d_inner half = c%2. The own half of d_inner is permuted FIRST in every
    d_inner-ordered tensor, so the device kernel is identical on all cores."""
    x = np.asarray(inputs["x"], dtype=np.float32)
    in_proj_w = np.asarray(inputs["in_proj_w"], dtype=np.float32)
    conv_w = np.asarray(inputs["conv_w"], dtype=np.float32)
    conv_b = np.asarray(inputs["conv_b"], dtype=np.float32)
    x_proj_w = np.asarray(inputs["x_proj_w"], dtype=np.float32)
    dt_proj_w = np.asarray(inputs["dt_proj_w"], dtype=np.float32)
    dt_proj_b = np.asarray(inputs["dt_proj_b"], dtype=np.float32)
    A = -np.exp(np.asarray(inputs["A_log"], dtype=np.float32))
    Dp = np.asarray(inputs["D"], dtype=np.float32)
    out_proj_w = np.asarray(inputs["out_proj_w"], dtype=np.float32)
    ln_g = np.asarray(inputs["ln_g"], dtype=np.float32)
    ln_b = np.asarray(inputs["ln_b"], dtype=np.float32)
    ff_w1 = np.asarray(inputs["ff_w1"], dtype=np.float32)
    ff_b1 = np.asarray(inputs["ff_b1"], dtype=np.float32)
    ff_w2 = np.asarray(inputs["ff_w2"], dtype=np.float32)
    ff_b2 = np.asarray(inputs["ff_b2"], dtype=np.float32)

    def cols(v):  # (N,) -> (P, N//P) per-partition column layout
        return np.ascontiguousarray(v.reshape(-1, P).T)

    def tile_w(w, KP, MP):  # (K, M) -> (K//KP, M//MP, KP, MP)
        K, M = w.shape
        return np.ascontiguousarray(
            w.reshape(K // KP, KP, M // MP, MP).transpose(0, 2, 1, 3))

    in_maps = []
    for c in range(8):
        s, half = c // 2, c % 2
        xb = x[s] if s < 2 else x[s - 2][::-1]
        perm = np.arange(D_INNER).reshape(2, HALF)
        perm = np.concatenate([perm[half], perm[1 - half]])
        own = perm[:HALF]

        wz = in_proj_w[:, D_INNER + own]                      # (1024, 1024)
        wxc = in_proj_w[:, perm]                              # (1024, 2048)
        w_in = np.concatenate([wz, wxc], axis=1)              # (1024, 3072)
        w_in_t = np.ascontiguousarray(tile_w(w_in, P, P).transpose(1, 0, 2, 3))

        cw = conv_w[perm]  # (2048, 4) -> (P, 16*4): col j*4+i = w[jP+p, i]
        convw_cols = np.ascontiguousarray(
            cw.reshape(NFB_XC, P, D_CONV).transpose(1, 0, 2).reshape(P, NFB_XC * D_CONV))

        g = (c & 1) + 2 * (c >> 2)
        hsl = slice(g * FF_SLICE, (g + 1) * FF_SLICE)

        in_maps.append({
            "xT": np.ascontiguousarray(xb.T),
            "w_in_t": w_in_t,
            "convw_cols": convw_cols,
            "convb_cols": cols(conv_b[perm]),
            "xpw_t": np.ascontiguousarray(
                np.concatenate([
                    x_proj_w[perm][:, :DT_RANK + D_STATE],
                    np.zeros((D_INNER, D_STATE), np.float32),
                    x_proj_w[perm][:, DT_RANK + D_STATE:],
                    np.zeros((D_INNER, D_STATE), np.float32),
                ], axis=1).reshape(NFB_XC, P, P)),
            "dtw_t": np.ascontiguousarray(
                dt_proj_w[:, own].reshape(DT_RANK, NJ, P).transpose(1, 0, 2)),
            "dtb_cols": cols(dt_proj_b[own]),
            "A_cols": np.ascontiguousarray(
                A[own].reshape(NJ, P, D_STATE).transpose(1, 0, 2).reshape(P, NJ * D_STATE)),
            "D_colsT": cols(Dp[own]),
            "outw_t": tile_w(out_proj_w[own], P, P),
            "lng_cols": cols(ln_g),
            "lnb_cols": cols(ln_b),
            "w1_t": tile_w(ff_w1[:, hsl], P, P),
            "b1_cols": cols(ff_b1[hsl]),
            "w2_t": tile_w(ff_w2[hsl], P, P),
            "b2_cols": cols(ff_b2[g * 256:(g + 1) * 256]),
            "ident_r": np.eye(P, dtype=np.float32),
            "consts_r": np.concatenate(
                [np.full((P, 1), 1.0 / D_MODEL, np.float32),
                 np.zeros((P, 3), np.float32)], axis=1),
        })
    return in_maps


_NC_CACHE = {}


def _get_nc():
    if "nc" not in _NC_CACHE:
        _NC_CACHE["nc"] = _build_nc()
    return _NC_CACHE["nc"]


def run(inputs, trace=False):
    _install_ntff_hook_shim()
    from concourse import bass_utils
    nc = _get_nc()
    in_maps = _prep_inputs(inputs)
    res = bass_utils.run_bass_kernel_spmd(nc, in_maps, core_ids=list(range(8)),
                                          trace=trace)
    # each core holds the dm-quarter (rows g*256..) of its group's output
    full = np.zeros((2, D_MODEL, L), np.float32)
    for c in range(8):
        b = 0 if c in (0, 1, 4, 5) else 1
        g = (c & 1) + 2 * (c >> 2)
        full[b, g * 256:(g + 1) * 256, :] = res.results[c]["out_m"]
    out = np.ascontiguousarray(full.transpose(0, 2, 1))
    return out, res


def kernel(**inputs):
    out, _ = run(inputs, trace=False)
    return out



# revision 7
# speedup vs baseline: 1.1740x; 1.1740x over previous
"""BiMamba (bidirectional Mamba block + LN + FFN) Trainium2 Bass kernel.

Sharding (8 cores): 4 scan-sequences (fwd/bwd x batch, bwd fed host-flipped x)
x 2 halves of d_inner. Feature-on-partitions / time-on-free throughout.

v2 layout/changes vs baseline:
 - all matmuls bf16 (weights converted on host), all scan elementwise in bf16
   (DVE 2x mode); scans partially offloaded to the gpsimd (Pool) engine.
 - in_proj computes only the core's own d_inner half of xc; the x_proj
   output (dt|B|C) partials are summed with a pairwise AllReduce.
 - delta (softplus of dt_proj) computed once and kept in SBUF.
 - direction merge + d_inner-half merge via ONE quad AllReduce of the
   out_proj partials; bwd cores pre-reverse their partial with a
   copy_predicated driven by a per-core mask input (program stays SPMD).
 - LN stats computed locally on the AllReduced mo (no extra collective).
 - back end (out_proj/LN/FFN) pipelined in two reversal-symmetric column
   chunks: A = cols [0:256)+[768:1024), B = cols [256:768), so collectives
   overlap compute.
"""
import sys, os, types, contextlib, ctypes

sys.path.insert(0, "/opt/trn_rl_repo")
import numpy as np
import ml_dtypes

BF16 = ml_dtypes.bfloat16

D_MODEL = 1024
D_STATE = 16
D_CONV = 4
D_INNER = 2048
DT_RANK = 64
L = 1024
HALF = D_INNER // 2          # 1024 d_inner per core
P = 128
NJ = HALF // P               # 8 d-blocks per core half
TCH = 512                    # matmul t-chunk
NT = L // TCH
KD = D_MODEL // P            # 8 k-chunks over d_model
FF_SLICE = 1024              # ffn hidden slice per core

_QUADS = [[0, 1, 4, 5], [2, 3, 6, 7]]
_PAIRS = [[0, 1], [2, 3], [4, 5], [6, 7]]

# column segments for the two reversal-symmetric back-end chunks
_HSEGS = [((0, 256), (768, 1024)), ((256, 512), (512, 768))]

# gpsimd (Pool) has no TensorScalarPtr (scan/STT) support; it can only run
# TensorTensor ops in software. Offload a share of the scan-phase multiplies.
POOL_B_NS = frozenset({0, 2, 4, 6, 8, 10, 12})   # b-mult on Pool for these n
POOL_P_NS = frozenset({1, 3, 5, 7, 9, 11, 13})   # prod-mult on Pool for these n


def _install_ntff_hook_shim(so_path="/opt/axon/libaxon_pjrt.so"):
    if "antenv.axon_hooks" in sys.modules:
        return
    try:
        lib = ctypes.CDLL(so_path)
    except OSError:
        return
    if not hasattr(lib, "axon_start_nrt_profile"):
        return
    lib.axon_start_nrt_profile.argtypes = [ctypes.POINTER(ctypes.c_int64), ctypes.c_size_t]
    lib.axon_start_nrt_profile.restype = ctypes.c_int64
    lib.axon_stop_nrt_profile.argtypes = [ctypes.c_char_p]
    lib.axon_stop_nrt_profile.restype = ctypes.c_int64

    @contextlib.contextmanager
    def _hook(output_dir, device_ids):
        import jax
        jax.devices()
        if device_ids:
            ids = (ctypes.c_int64 * len(device_ids))(*device_ids)
            rc = lib.axon_start_nrt_profile(ids, len(device_ids))
        else:
            rc = lib.axon_start_nrt_profile(None, 0)
        if rc != 0:
            raise RuntimeError(f"axon_start_nrt_profile rc={rc}")
        try:
            yield
        finally:
            n = lib.axon_stop_nrt_profile(str(output_dir).encode())
            print(f"profile: {n} file(s) written to {output_dir}", file=sys.stderr)

    mod = types.ModuleType("antenv.axon_hooks")
    mod.get_axon_ntff_profile_hook = lambda: _hook
    mod.set_axon_ntff_profile_hook = lambda h: None
    sys.modules["antenv.axon_hooks"] = mod


def _build_nc():
    from concourse import bacc, tile, mybir

    f32 = mybir.dt.float32
    bf16 = mybir.dt.bfloat16
    u8 = mybir.dt.uint8
    Alu = mybir.AluOpType
    Act = mybir.ActivationFunctionType

    nc = bacc.Bacc("TRN2", target_bir_lowering=False, debug=False, num_devices=8)

    def din(name, shape, dt=None):
        return nc.dram_tensor(name, list(shape), dt or f32, kind="ExternalInput").ap()

    xT = din("xT", (D_MODEL, L), bf16)
    w_in_t = din("w_in_t", (2 * NJ, KD, P, P), bf16)     # fb 0..7 xc-half, 8..15 z-half
    convw_cols = din("convw_cols", (P, NJ * D_CONV))
    convb_cols = din("convb_cols", (P, NJ))
    xpw_t = din("xpw_t", (NJ, P, P), bf16)               # cols: dt64|B16|pad|C16|pad
    dtw_t = din("dtw_t", (NJ, DT_RANK, P), bf16)
    dtb_cols = din("dtb_cols", (P, NJ))
    A_cols = din("A_cols", (P, NJ * D_STATE))
    D_colsT = din("D_colsT", (P, NJ))
    outw_t = din("outw_t", (NJ, KD, P, P), bf16)         # [k(own d_in), m(dm)]
    lng_cols = din("lng_cols", (P, KD))
    lnb_cols = din("lnb_cols", (P, KD))
    w1_t = din("w1_t", (KD, NJ, P, P), bf16)             # [k(dm), m(h)]
    b1_cols = din("b1_cols", (P, NJ))
    w2_t = din("w2_t", (NJ, KD, P, P), bf16)             # [k(h), m(dm)]
    b2_cols = din("b2_cols", (P, 2))
    ident_b = din("ident_b", (P, P), bf16)
    dirmask = din("dirmask", (P, TCH), u8)               # 1 on bwd cores
    onescol = din("onescol", (P, 1), bf16)               # 2^-10 (1/1024)
    onesrow = din("onesrow", (1, P), bf16)               # 1.0

    out_m = nc.dram_tensor("out_m", [D_MODEL // 4, L], f32, kind="ExternalOutput").ap()

    with tile.TileContext(nc) as tc:
        with contextlib.ExitStack() as stk:
            cpool = stk.enter_context(tc.tile_pool(name="cpool", bufs=1))
            dram = stk.enter_context(tc.tile_pool(name="dram", bufs=1, space="DRAM"))

            def cload(src, shape, tag, dt=f32):
                t = cpool.tile(list(shape), dt, tag=tag, name=tag)
                nc.sync.dma_start(t[:], src)
                return t

            A_sb = cload(A_cols[:], (P, NJ * D_STATE), "A_sb")
            dtb_sb = cload(dtb_cols[:], (P, NJ), "dtb_sb")
            D_sb = cload(D_colsT[:], (P, NJ), "D_sb")
            convb_sb = cload(convb_cols[:], (P, NJ), "convb_sb")
            convw_sb = cload(convw_cols[:], (P, NJ * D_CONV), "convw_sb")
            lng_sb = cload(lng_cols[:], (P, KD), "lng_sb")
            lnb_sb = cload(lnb_cols[:], (P, KD), "lnb_sb")
            b1_sb = cload(b1_cols[:], (P, NJ), "b1_sb")
            b2_sb = cload(b2_cols[:], (P, 2), "b2_sb")
            ident_sb = cload(ident_b[:], (P, P), "ident_sb", bf16)
            dirmask_sb = cload(dirmask[:], (P, TCH), "dirmask_sb", u8)
            onescol_sb = cload(onescol[:], (P, 1), "onescol_sb", bf16)
            onesrow_sb = cload(onesrow[:], (1, P), "onesrow_sb", bf16)

            dbl_in_d = dram.tile([P, L], bf16, name="dbl_in_d")
            dbl_out_d = dram.tile([P, L], bf16, name="dbl_out_d")
            bcB_d = dram.tile([D_STATE, L], bf16, name="bcB_d")
            bcC_d = dram.tile([D_STATE, L], bf16, name="bcC_d")
            arm_in = [dram.tile([D_MODEL, TCH], bf16, name=f"arm_in{h}") for h in range(2)]
            arm_out = [dram.tile([D_MODEL, TCH], bf16, name=f"arm_out{h}") for h in range(2)]
            ar2_in = [dram.tile([D_MODEL, TCH], bf16, name=f"ar2_in{h}") for h in range(2)]
            rs2_out = [dram.tile([D_MODEL // 4, TCH], bf16, name=f"rs2_out{h}") for h in range(2)]

            def mm_accum(ps, lw_list, rhs_of_k, n_k):
                for k in range(n_k):
                    nc.tensor.matmul(ps[:], lw_list[k][:], rhs_of_k(k),
                                     start=(k == 0), stop=(k == n_k - 1))

            # persistent across the scan
            per_pool = stk.enter_context(tc.tile_pool(name="per_pool", bufs=1))
            sz = [per_pool.tile([P, L], bf16, tag=f"sz{j}", name=f"sz{j}")
                  for j in range(NJ)]
            wvs = [per_pool.tile([P, L], bf16, tag=f"wv{j}", name=f"wv{j}")
                   for j in range(NJ)]
            g0s = [per_pool.tile([P, L], bf16, tag=f"g0{j}", name=f"g0{j}")
                   for j in range(NJ)]
            deltas = [per_pool.tile([P, L], bf16, tag=f"delta{j}", name=f"delta{j}")
                      for j in range(NJ)]
            ygs = [per_pool.tile([P, L], bf16, tag=f"yg{j}", name=f"yg{j}")
                   for j in range(NJ)]

            # ---------------- P1..P4: produce xc, sz, delta, wv, g0 ----------------
            with tc.tile_pool(name="xc_pool", bufs=1) as xc_pool, \
                 tc.tile_pool(name="xt_pool", bufs=1) as xt_pool, \
                 tc.tile_pool(name="p1t", bufs=1) as p1t, \
                 tc.tile_pool(name="psumA", bufs=4, space="PSUM") as psumA:
                xcs = [xc_pool.tile([P, L], bf16, tag=f"xcs{j}", name=f"xcs{j}")
                       for j in range(NJ)]
                xts = []
                for k in range(KD):
                    xt_k = xt_pool.tile([P, L], bf16, tag=f"xt{k}", name=f"xt{k}")
                    nc.sync.dma_start(xt_k[:], xT[k * P:(k + 1) * P, :])
                    xts.append(xt_k)

                def in_proj_block(fb):
                    lws = []
                    for k in range(KD):
                        lw = p1t.tile([P, P], bf16, tag=f"lw{k}",
                                      name=f"lw{fb}_{k}", bufs=2)
                        nc.sync.dma_start(lw[:], w_in_t[fb, k])
                        lws.append(lw)
                    pss = []
                    for t in range(NT):
                        ps = psumA.tile([P, TCH], f32, tag="ps", name=f"inp{fb}_{t}")
                        mm_accum(ps, lws,
                                 lambda k: xts[k][:, t * TCH:(t + 1) * TCH], KD)
                        pss.append(ps)
                    return pss

                # P1: xc half + conv + silu
                for j in range(NJ):
                    xcp = p1t.tile([P, L + D_CONV - 1], bf16, tag="xcp",
                                   name=f"xcp{j}", bufs=2)
                    nc.vector.memset(xcp[:, 0:D_CONV - 1], 0.0)
                    for t, ps in enumerate(in_proj_block(j)):
                        nc.scalar.copy(
                            xcp[:, D_CONV - 1 + t * TCH:D_CONV - 1 + (t + 1) * TCH],
                            ps[:])
                    cacc = p1t.tile([P, L], bf16, tag="cacc", name=f"cacc{j}", bufs=2)
                    nc.vector.tensor_scalar_mul(
                        cacc[:], xcp[:, 0:L],
                        convw_sb[:, j * D_CONV:j * D_CONV + 1])
                    for i in range(1, D_CONV):
                        nc.vector.scalar_tensor_tensor(
                            cacc[:], xcp[:, i:i + L],
                            convw_sb[:, j * D_CONV + i:j * D_CONV + i + 1],
                            cacc[:], Alu.mult, Alu.add)
                    nc.scalar.activation(xcs[j][:], cacc[:], Act.Silu,
                                         bias=convb_sb[:, j:j + 1])

                # P2: x_proj partial over own xc half -> pairwise AllReduce
                dblp = p1t.tile([P, L], bf16, tag="dblp", name="dblp", bufs=1)
                for t in range(NT):
                    ps = psumA.tile([P, TCH], f32, tag="ps", name=f"xproj{t}")
                    for k in range(NJ):
                        lw = p1t.tile([P, P], bf16, tag="xpw", name=f"xpw{t}_{k}",
                                      bufs=2)
                        nc.sync.dma_start(lw[:], xpw_t[k])
                        nc.tensor.matmul(ps[:], lw[:],
                                         xcs[k][:, t * TCH:(t + 1) * TCH],
                                         start=(k == 0), stop=(k == NJ - 1))
                    nc.scalar.copy(dblp[:, t * TCH:(t + 1) * TCH], ps[:])
                nc.sync.dma_start(dbl_in_d[:], dblp[:])
                nc.gpsimd.collective_compute(
                    "AllReduce", Alu.add, replica_groups=_PAIRS,
                    ins=[dbl_in_d[:]], outs=[dbl_out_d[:]])

                # P3 (overlaps the pair AllReduce): z half in_proj + silu, g0
                for j in range(NJ):
                    for t, ps in enumerate(in_proj_block(NJ + j)):
                        nc.scalar.activation(sz[j][:, t * TCH:(t + 1) * TCH],
                                             ps[:], Act.Silu)
                for j in range(NJ):
                    tg = p1t.tile([P, L], bf16, tag="tg", name=f"tg{j}", bufs=2)
                    nc.vector.tensor_scalar_mul(tg[:], xcs[j][:], D_sb[:, j:j + 1])
                    nc.vector.tensor_tensor(g0s[j][:], tg[:], sz[j][:], Alu.mult)

                # P4: dt_proj + softplus -> delta; wv; broadcast B/C rows
                dbl_sb = p1t.tile([P, L], bf16, tag="dbl_sb", name="dbl_sb", bufs=1)
                nc.sync.dma_start(dbl_sb[:], dbl_out_d[:])
                for j in range(NJ):
                    lw = p1t.tile([DT_RANK, P], bf16, tag="dtw", name=f"dtw{j}",
                                  bufs=2)
                    nc.sync.dma_start(lw[:], dtw_t[j])
                    for t in range(NT):
                        ps = psumA.tile([P, TCH], f32, tag="ps", name=f"dtp{j}_{t}")
                        nc.tensor.matmul(ps[:], lw[:],
                                         dbl_sb[0:DT_RANK, t * TCH:(t + 1) * TCH],
                                         start=True, stop=True)
                        spt = p1t.tile([P, TCH], bf16, tag="spt",
                                       name=f"spt{j}_{t}", bufs=2)
                        nc.scalar.activation(spt[:], ps[:], Act.Exp,
                                             bias=dtb_sb[:, j:j + 1])
                        nc.scalar.activation(
                            deltas[j][:, t * TCH:(t + 1) * TCH], spt[:],
                            Act.Ln, bias=1.0)
                    nc.vector.tensor_tensor(wvs[j][:], deltas[j][:], xcs[j][:],
                                            Alu.mult)
                bcB_sb = p1t.tile([D_STATE, L], bf16, tag="bcB_sb", name="bcB_sb",
                                  bufs=1)
                nc.scalar.copy(bcB_sb[:], dbl_sb[DT_RANK:DT_RANK + D_STATE, :])
                nc.sync.dma_start(bcB_d[:], bcB_sb[:])
                bcC_sb = p1t.tile([D_STATE, L], bf16, tag="bcC_sb", name="bcC_sb",
                                  bufs=1)
                nc.scalar.copy(bcC_sb[:], dbl_sb[96:96 + D_STATE, :])
                nc.sync.dma_start(bcC_d[:], bcC_sb[:])

            # ---------------- P5: scan (4 j-blocks per psum wave) ----------------
            with tc.tile_pool(name="tpool", bufs=1) as tpool, \
                 tc.tile_pool(name="pscan", bufs=1, space="PSUM") as pscan:
                for hb in range(2):
                    js = list(range(hb * 4, hb * 4 + 4))
                    yps = {j: pscan.tile([P, L], f32, tag=f"yps{j % 4}",
                                         name=f"yps{j}") for j in js}
                    for n in range(D_STATE):
                        Bbc = tpool.tile([P, L], bf16, tag="Bbc",
                                         name=f"Bbc{hb}_{n}", bufs=3)
                        nc.sync.dma_start(
                            Bbc[:],
                            bcB_d[n:n + 1, :].partition_broadcast(P).squeeze(1))
                        Cbc = tpool.tile([P, L], bf16, tag="Cbc",
                                         name=f"Cbc{hb}_{n}", bufs=3)
                        nc.sync.dma_start(
                            Cbc[:],
                            bcC_d[n:n + 1, :].partition_broadcast(P).squeeze(1))
                        beng = nc.gpsimd if n in POOL_B_NS else nc.vector
                        peng = nc.gpsimd if n in POOL_P_NS else nc.vector
                        for j in js:
                            a_t = tpool.tile([P, L], bf16, tag="a_t",
                                             name=f"a{j}_{n}", bufs=3)
                            nc.scalar.activation(
                                a_t[:], deltas[j][:], Act.Exp,
                                scale=A_sb[:, j * D_STATE + n:j * D_STATE + n + 1])
                            b_t = tpool.tile([P, L], bf16, tag="b_t",
                                             name=f"b{j}_{n}", bufs=2)
                            beng.tensor_tensor(b_t[:], wvs[j][:], Bbc[:],
                                               Alu.mult)
                            h_t = tpool.tile([P, L], bf16, tag="h_t",
                                             name=f"h{j}_{n}", bufs=2)
                            nc.vector.tensor_tensor_scan(h_t[:], a_t[:], b_t[:],
                                                          0.0, Alu.mult, Alu.add)
                            prod = tpool.tile([P, L], bf16, tag="prod",
                                              name=f"p{j}_{n}", bufs=3)
                            peng.tensor_tensor(prod[:], h_t[:], Cbc[:],
                                               Alu.mult)
                            for t in range(NT):
                                sl = slice(t * TCH, (t + 1) * TCH)
                                nc.tensor.matmul(yps[j][:, sl], ident_sb[:],
                                                 prod[:, sl],
                                                 start=(n == 0),
                                                 stop=(n == D_STATE - 1))
                    for j in js:
                        yb = tpool.tile([P, L], bf16, tag="yb", name=f"yb{j}",
                                        bufs=2)
                        nc.scalar.copy(yb[:], yps[j][:])
                        ygt = tpool.tile([P, L], bf16, tag="ygt", name=f"ygt{j}",
                                         bufs=2)
                        nc.vector.tensor_tensor(ygt[:], yb[:], sz[j][:], Alu.mult)
                        nc.vector.tensor_tensor(ygs[j][:], ygt[:], g0s[j][:],
                                                Alu.add)

            # ---------------- P6..P8: out_proj + AR, LN, FFN per column chunk ----
            with tc.tile_pool(name="p6t", bufs=1) as p6t, \
                 tc.tile_pool(name="psumB", bufs=4, space="PSUM") as psumB, \
                 tc.tile_pool(name="pstat", bufs=1, space="PSUM") as pstat:

                def out_proj_half(ha):
                    segs = _HSEGS[ha]
                    for m in range(NJ):
                        lws = []
                        for k in range(NJ):
                            lw = p6t.tile([P, P], bf16, tag=f"olw{k}",
                                          name=f"ow{ha}_{m}_{k}", bufs=2)
                            nc.sync.dma_start(lw[:], outw_t[k, m])
                            lws.append(lw)
                        ps = psumB.tile([P, TCH], f32, tag="ps", name=f"op{ha}_{m}")
                        for ci, (c0, c1) in enumerate(segs):
                            for k in range(NJ):
                                nc.tensor.matmul(
                                    ps[:, ci * 256:(ci + 1) * 256], lws[k][:],
                                    ygs[k][:, c0:c1],
                                    start=(k == 0), stop=(k == NJ - 1))
                        msb = p6t.tile([P, TCH], bf16, tag="msb",
                                       name=f"msb{ha}_{m}", bufs=2)
                        nc.scalar.copy(msb[:], ps[:])
                        nc.vector.copy_predicated(msb[:], dirmask_sb[:],
                                                  ps[:, ::-1])
                        nc.sync.dma_start(arm_in[ha][m * P:(m + 1) * P, :], msb[:])
                    nc.gpsimd.collective_compute(
                        "AllReduce", Alu.add, replica_groups=_QUADS,
                        ins=[arm_in[ha][:]], outs=[arm_out[ha][:]])

                def ln_ffn_half(ha):
                    segs = _HSEGS[ha]
                    mos = []
                    mu_ps = pstat.tile([1, TCH], f32, tag="mu_ps",
                                       name=f"mu{ha}", bufs=1)
                    e2_ps = pstat.tile([1, TCH], f32, tag="e2_ps",
                                       name=f"e2{ha}", bufs=1)
                    for m in range(KD):
                        mo = p6t.tile([P, TCH], bf16, tag=f"mo{m}",
                                      name=f"mo{ha}_{m}", bufs=1)
                        nc.sync.dma_start(mo[:], arm_out[ha][m * P:(m + 1) * P, :])
                        mos.append(mo)
                        sq = p6t.tile([P, TCH], bf16, tag="sq", name=f"sq{ha}_{m}",
                                      bufs=2)
                        nc.scalar.activation(sq[:], mo[:], Act.Square)
                        nc.tensor.matmul(mu_ps[:], onescol_sb[:], mo[:],
                                         start=(m == 0), stop=(m == KD - 1))
                        nc.tensor.matmul(e2_ps[:], onescol_sb[:], sq[:],
                                         start=(m == 0), stop=(m == KD - 1))
                    m2 = p6t.tile([1, TCH], f32, tag="m2", name=f"m2{ha}", bufs=1)
                    nc.scalar.activation(m2[:], mu_ps[:], Act.Square)
                    var_t = p6t.tile([1, TCH], f32, tag="var_t", name=f"var{ha}",
                                     bufs=1)
                    nc.vector.tensor_tensor(var_t[:], e2_ps[:], m2[:],
                                            Alu.subtract)
                    eps_sb = p6t.tile([1, 1], f32, tag="eps_sb", name=f"eps{ha}",
                                      bufs=1)
                    nc.vector.memset(eps_sb[:], 1e-5)
                    std_t = p6t.tile([1, TCH], f32, tag="std_t", name=f"std{ha}",
                                     bufs=1)
                    nc.scalar.activation(std_t[:], var_t[:], Act.Sqrt,
                                         bias=eps_sb[:])
                    rstd_b = p6t.tile([1, TCH], bf16, tag="rstd_b",
                                      name=f"rstd{ha}", bufs=1)
                    with nc.allow_low_precision(reason="bf16 rstd for broadcast"):
                        nc.vector.reciprocal(rstd_b[:], std_t[:])
                    mean_b = p6t.tile([1, TCH], bf16, tag="mean_b",
                                      name=f"mean{ha}", bufs=1)
                    nc.scalar.copy(mean_b[:], mu_ps[:])
                    mean_ps = pstat.tile([P, TCH], f32, tag="mean_ps",
                                         name=f"meanbc{ha}", bufs=1)
                    nc.tensor.matmul(mean_ps[:], onesrow_sb[:], mean_b[:],
                                     start=True, stop=True)
                    rstd_ps = pstat.tile([P, TCH], f32, tag="rstd_ps",
                                         name=f"rstdbc{ha}", bufs=1)
                    nc.tensor.matmul(rstd_ps[:], onesrow_sb[:], rstd_b[:],
                                     start=True, stop=True)
                    mean_bc = p6t.tile([P, TCH], bf16, tag="mean_bc",
                                       name=f"meanbcs{ha}", bufs=1)
                    nc.scalar.copy(mean_bc[:], mean_ps[:])
                    rstd_bc = p6t.tile([P, TCH], bf16, tag="rstd_bc",
                                       name=f"rstdbcs{ha}", bufs=1)
                    nc.scalar.copy(rstd_bc[:], rstd_ps[:])

                    xns = []
                    for m in range(KD):
                        t1 = p6t.tile([P, TCH], bf16, tag="lnt", name=f"lnt{ha}_{m}",
                                      bufs=2)
                        nc.vector.tensor_tensor(t1[:], mos[m][:], mean_bc[:],
                                                Alu.subtract)
                        nc.vector.tensor_tensor(t1[:], t1[:], rstd_bc[:], Alu.mult)
                        xn = p6t.tile([P, TCH], bf16, tag=f"xn{m}",
                                      name=f"xn{ha}_{m}", bufs=1)
                        nc.vector.tensor_scalar(xn[:], t1[:], lng_sb[:, m:m + 1],
                                                lnb_sb[:, m:m + 1], Alu.mult,
                                                Alu.add)
                        xns.append(xn)

                    ffhs = []
                    for m in range(NJ):
                        lws = []
                        for k in range(KD):
                            lw = p6t.tile([P, P], bf16, tag=f"flw{k}",
                                          name=f"w1_{ha}_{m}_{k}", bufs=2)
                            nc.sync.dma_start(lw[:], w1_t[k, m])
                            lws.append(lw)
                        ps = psumB.tile([P, TCH], f32, tag="ps", name=f"f1{ha}_{m}")
                        mm_accum(ps, lws, lambda k: xns[k][:], KD)
                        ffh = p6t.tile([P, TCH], bf16, tag=f"ffh{m}",
                                       name=f"ffh{ha}_{m}", bufs=1)
                        nc.scalar.activation(ffh[:], ps[:], Act.Gelu,
                                             bias=b1_sb[:, m:m + 1])
                        ffhs.append(ffh)
                    for m in range(KD):
                        lws = []
                        for k in range(NJ):
                            lw = p6t.tile([P, P], bf16, tag=f"flw{k}",
                                          name=f"w2_{ha}_{m}_{k}", bufs=2)
                            nc.sync.dma_start(lw[:], w2_t[k, m])
                            lws.append(lw)
                        ps = psumB.tile([P, TCH], f32, tag="ps", name=f"f2{ha}_{m}")
                        mm_accum(ps, lws, lambda k: ffhs[k][:], NJ)
                        f2 = p6t.tile([P, TCH], bf16, tag="f2", name=f"f2{ha}_{m}",
                                      bufs=2)
                        nc.scalar.copy(f2[:], ps[:])
                        nc.sync.dma_start(ar2_in[ha][m * P:(m + 1) * P, :], f2[:])
                    nc.gpsimd.collective_compute(
                        "ReduceScatter", Alu.add, replica_groups=_QUADS,
                        ins=[ar2_in[ha][:]], outs=[rs2_out[ha][:]])

                def store_half(ha):
                    segs = _HSEGS[ha]
                    for q in range(2):
                        fin = p6t.tile([P, TCH], bf16, tag="fin", name=f"fin{ha}_{q}",
                                       bufs=2)
                        nc.sync.dma_start(fin[:], rs2_out[ha][q * P:(q + 1) * P, :])
                        fob = p6t.tile([P, TCH], f32, tag="fob", name=f"fob{ha}_{q}",
                                       bufs=2)
                        nc.vector.tensor_scalar_add(fob[:], fin[:],
                                                    b2_sb[:, q:q + 1])
                        for ci, (c0, c1) in enumerate(segs):
                            nc.sync.dma_start(
                                out_m[q * P:(q + 1) * P, c0:c1],
                                fob[:, ci * 256:(ci + 1) * 256])

                out_proj_half(0)
                out_proj_half(1)
                ln_ffn_half(0)
                ln_ffn_half(1)
                store_half(0)
                store_half(1)

    nc.compile()
    return nc


def _prep_inputs(inputs):
    """Per-core input dicts. Core c: sequence s=c//2 (s>=2 => time-flipped x),
    d_inner half = c%2. The own half of d_inner is permuted FIRST in every
    d_inner-ordered tensor, so the device kernel is identical on all cores."""
    x = np.asarray(inputs["x"], dtype=np.float32)
    in_proj_w = np.asarray(inputs["in_proj_w"], dtype=np.float32)
    conv_w = np.asarray(inputs["conv_w"], dtype=np.float32)
    conv_b = np.asarray(inputs["conv_b"], dtype=np.float32)
    x_proj_w = np.asarray(inputs["x_proj_w"], dtype=np.float32)
    dt_proj_w = np.asarray(inputs["dt_proj_w"], dtype=np.float32)
    dt_proj_b = np.asarray(inputs["dt_proj_b"], dtype=np.float32)
    A = -np.exp(np.asarray(inputs["A_log"], dtype=np.float32))
    Dp = np.asarray(inputs["D"], dtype=np.float32)
    out_proj_w = np.asarray(inputs["out_proj_w"], dtype=np.float32)
    ln_g = np.asarray(inputs["ln_g"], dtype=np.float32)
    ln_b = np.asarray(inputs["ln_b"], dtype=np.float32)
    ff_w1 = np.asarray(inputs["ff_w1"], dtype=np.float32)
    ff_b1 = np.asarray(inputs["ff_b1"], dtype=np.float32)
    ff_w2 = np.asarray(inputs["ff_w2"], dtype=np.float32)
    ff_b2 = np.asarray(inputs["ff_b2"], dtype=np.float32)

    def cols(v):  # (N,) -> (P, N//P) per-partition column layout
        return np.ascontiguousarray(v.reshape(-1, P).T)

    def tile_w(w, KP, MP):  # (K, M) -> (K//KP, M//MP, KP, MP) bf16
        K, M = w.shape
        return np.ascontiguousarray(
            w.reshape(K // KP, KP, M // MP, MP).transpose(0, 2, 1, 3)
        ).astype(BF16)

    in_maps = []
    for c in range(8):
        s, half = c // 2, c % 2
        xb = x[s] if s < 2 else x[s - 2][::-1]
        perm = np.arange(D_INNER).reshape(2, HALF)
        own = np.concatenate([perm[half], perm[1 - half]])[:HALF]

        wxc = in_proj_w[:, own]                               # (1024, 1024)
        wz = in_proj_w[:, D_INNER + own]                      # (1024, 1024)
        w_in = np.concatenate([wxc, wz], axis=1)              # (1024, 2048)
        w_in_t = np.ascontiguousarray(
            tile_w(w_in, P, P).transpose(1, 0, 2, 3))         # (16 fb, 8 k, P, P)

        cw = conv_w[own]  # (1024, 4) -> (P, 8*4): col j*4+i = w[jP+p, i]
        convw_cols = np.ascontiguousarray(
            cw.reshape(NJ, P, D_CONV).transpose(1, 0, 2).reshape(P, NJ * D_CONV))

        g = (c & 1) + 2 * (c >> 2)
        hsl = slice(g * FF_SLICE, (g + 1) * FF_SLICE)

        in_maps.append({
            "xT": np.ascontiguousarray(xb.T).astype(BF16),
            "w_in_t": w_in_t,
            "convw_cols": convw_cols,
            "convb_cols": cols(conv_b[own]),
            "xpw_t": np.ascontiguousarray(
                np.concatenate([
                    x_proj_w[own][:, :DT_RANK + D_STATE],
                    np.zeros((HALF, D_STATE), np.float32),
                    x_proj_w[own][:, DT_RANK + D_STATE:],
                    np.zeros((HALF, D_STATE), np.float32),
                ], axis=1).reshape(NJ, P, P)).astype(BF16),
            "dtw_t": np.ascontiguousarray(
                dt_proj_w[:, own].reshape(DT_RANK, NJ, P).transpose(1, 0, 2)
            ).astype(BF16),
            "dtb_cols": cols(dt_proj_b[own]),
            "A_cols": np.ascontiguousarray(
                A[own].reshape(NJ, P, D_STATE).transpose(1, 0, 2).reshape(
                    P, NJ * D_STATE)),
            "D_colsT": cols(Dp[own]),
            "outw_t": tile_w(out_proj_w[own], P, P),
            "lng_cols": cols(ln_g),
            "lnb_cols": cols(ln_b),
            "w1_t": tile_w(ff_w1[:, hsl], P, P),
            "b1_cols": cols(ff_b1[hsl]),
            "w2_t": tile_w(ff_w2[hsl], P, P),
            "b2_cols": cols(ff_b2[g * 256:(g + 1) * 256]),
            "ident_b": np.eye(P, dtype=np.float32).astype(BF16),
            "dirmask": np.full((P, TCH), 1 if s >= 2 else 0, np.uint8),
            "onescol": np.full((P, 1), 1.0 / 1024.0, np.float32).astype(BF16),
            "onesrow": np.ones((1, P), np.float32).astype(BF16),
        })
    return in_maps


_NC_CACHE = {}


def _get_nc():
    if "nc" not in _NC_CACHE:
        _NC_CACHE["nc"] = _build_nc()
    return _NC_CACHE["nc"]


def run(inputs, trace=False):
    _install_ntff_hook_shim()
    from concourse import bass_utils
    nc = _get_nc()
    in_maps = _prep_inputs(inputs)
    res = bass_utils.run_bass_kernel_spmd(nc, in_maps, core_ids=list(range(8)),
                                          trace=trace)
    # each core holds the dm-quarter (rows g*256..) of its group's output
    full = np.zeros((2, D_MODEL, L), np.float32)
    for c in range(8):
        b = 0 if c in (0, 1, 4, 5) else 1
        g = (c & 1) + 2 * (c >> 2)
        full[b, g * 256:(g + 1) * 256, :] = res.results[c]["out_m"]
    out = np.ascontiguousarray(full.transpose(0, 2, 1))
    return out, res


def kernel(**inputs):
    out, _ = run(inputs, trace=False)
    return out


# revision 10
# speedup vs baseline: 1.3533x; 1.1528x over previous
"""BiMamba (bidirectional Mamba block + LN + FFN) Trainium2 Bass kernel.

Sharding (8 cores): 4 scan-sequences (fwd/bwd x batch, bwd fed host-flipped x)
x 2 halves of d_inner. Feature-on-partitions / time-on-free throughout.

v2 layout/changes vs baseline:
 - all matmuls bf16 (weights converted on host), all scan elementwise in bf16
   (DVE 2x mode); scans partially offloaded to the gpsimd (Pool) engine.
 - in_proj computes only the core's own d_inner half of xc; the x_proj
   output (dt|B|C) partials are summed with a pairwise AllReduce.
 - delta (softplus of dt_proj) computed once and kept in SBUF.
 - direction merge + d_inner-half merge via ONE quad AllReduce of the
   out_proj partials; bwd cores pre-reverse their partial with a
   copy_predicated driven by a per-core mask input (program stays SPMD).
 - LN stats computed locally on the AllReduced mo (no extra collective).
 - back end (out_proj/LN/FFN) pipelined in two reversal-symmetric column
   chunks: A = cols [0:256)+[768:1024), B = cols [256:768), so collectives
   overlap compute.
"""
import sys, os, types, contextlib, ctypes

sys.path.insert(0, "/opt/trn_rl_repo")
import numpy as np
import ml_dtypes

BF16 = ml_dtypes.bfloat16

D_MODEL = 1024
D_STATE = 16
D_CONV = 4
D_INNER = 2048
DT_RANK = 64
L = 1024
HALF = D_INNER // 2          # 1024 d_inner per core
P = 128
NJ = HALF // P               # 8 d-blocks per core half
TCH = 512                    # matmul t-chunk
NT = L // TCH
KD = D_MODEL // P            # 8 k-chunks over d_model
FF_SLICE = 1024              # ffn hidden slice per core

_QUADS = [[0, 1, 4, 5], [2, 3, 6, 7]]
_PAIRS = [[0, 1], [2, 3], [4, 5], [6, 7]]

# column segments for the two reversal-symmetric back-end chunks
_HSEGS = [((0, 256), (768, 1024)), ((256, 512), (512, 768))]

# gpsimd (Pool) has no TensorScalarPtr (scan/STT) support; it can only run
# TensorTensor ops in software. Offload a share of the scan-phase multiplies.
POOL_B_NS = frozenset({0, 2, 4, 6, 8, 10, 12})   # b-mult on Pool for these n
POOL_P_NS = frozenset({1, 3, 5, 7, 9, 11, 13})   # prod-mult on Pool for these n


def _install_ntff_hook_shim(so_path="/opt/axon/libaxon_pjrt.so"):
    if "antenv.axon_hooks" in sys.modules:
        return
    try:
        lib = ctypes.CDLL(so_path)
    except OSError:
        return
    if not hasattr(lib, "axon_start_nrt_profile"):
        return
    lib.axon_start_nrt_profile.argtypes = [ctypes.POINTER(ctypes.c_int64), ctypes.c_size_t]
    lib.axon_start_nrt_profile.restype = ctypes.c_int64
    lib.axon_stop_nrt_profile.argtypes = [ctypes.c_char_p]
    lib.axon_stop_nrt_profile.restype = ctypes.c_int64

    @contextlib.contextmanager
    def _hook(output_dir, device_ids):
        import jax
        jax.devices()
        if device_ids:
            ids = (ctypes.c_int64 * len(device_ids))(*device_ids)
            rc = lib.axon_start_nrt_profile(ids, len(device_ids))
        else:
            rc = lib.axon_start_nrt_profile(None, 0)
        if rc != 0:
            raise RuntimeError(f"axon_start_nrt_profile rc={rc}")
        try:
            yield
        finally:
            n = lib.axon_stop_nrt_profile(str(output_dir).encode())
            print(f"profile: {n} file(s) written to {output_dir}", file=sys.stderr)

    mod = types.ModuleType("antenv.axon_hooks")
    mod.get_axon_ntff_profile_hook = lambda: _hook
    mod.set_axon_ntff_profile_hook = lambda h: None
    sys.modules["antenv.axon_hooks"] = mod


def _build_nc():
    from concourse import bacc, tile, mybir

    f32 = mybir.dt.float32
    bf16 = mybir.dt.bfloat16
    u8 = mybir.dt.uint8
    Alu = mybir.AluOpType
    Act = mybir.ActivationFunctionType

    nc = bacc.Bacc("TRN2", target_bir_lowering=False, debug=False, num_devices=8)

    def din(name, shape, dt=None):
        return nc.dram_tensor(name, list(shape), dt or f32, kind="ExternalInput").ap()

    xT = din("xT", (D_MODEL, L), bf16)
    w_in_t = din("w_in_t", (2 * NJ, KD, P, P), bf16)     # fb 0..7 xc-half, 8..15 z-half
    convw_cols = din("convw_cols", (P, NJ * D_CONV))
    convb_cols = din("convb_cols", (P, NJ))
    xpw_t = din("xpw_t", (NJ, P, P), bf16)               # cols: dt64|B16|pad|C16|pad
    dtw_t = din("dtw_t", (NJ, DT_RANK, P), bf16)
    dtb_cols = din("dtb_cols", (P, NJ))
    A_cols = din("A_cols", (P, NJ * D_STATE))
    D_colsT = din("D_colsT", (P, NJ))
    outw_t = din("outw_t", (NJ, KD, P, P), bf16)         # [k(own d_in), m(dm)]
    lng_cols = din("lng_cols", (P, KD))
    lnb_cols = din("lnb_cols", (P, KD))
    w1_t = din("w1_t", (KD, NJ, P, P), bf16)             # [k(dm), m(h)]
    b1_cols = din("b1_cols", (P, NJ))
    w2_t = din("w2_t", (NJ, KD, P, P), bf16)             # [k(h), m(dm)]
    b2_cols = din("b2_cols", (P, 2))
    ident_b = din("ident_b", (P, P), bf16)
    dirmask = din("dirmask", (P, TCH), u8)               # 1 on bwd cores
    onescol = din("onescol", (P, 1), bf16)               # 2^-10 (1/1024)
    onesrow = din("onesrow", (1, P), bf16)               # 1.0

    out_m = nc.dram_tensor("out_m", [D_MODEL // 4, L], f32, kind="ExternalOutput").ap()

    with tile.TileContext(nc) as tc:
        with contextlib.ExitStack() as stk:
            cpool = stk.enter_context(tc.tile_pool(name="cpool", bufs=1))
            dram = stk.enter_context(tc.tile_pool(name="dram", bufs=1, space="DRAM"))

            def cload(src, shape, tag, dt=f32):
                t = cpool.tile(list(shape), dt, tag=tag, name=tag)
                nc.sync.dma_start(t[:], src)
                return t

            A_sb = cload(A_cols[:], (P, NJ * D_STATE), "A_sb")
            dtb_sb = cload(dtb_cols[:], (P, NJ), "dtb_sb")
            D_sb = cload(D_colsT[:], (P, NJ), "D_sb")
            convb_sb = cload(convb_cols[:], (P, NJ), "convb_sb")
            convw_sb = cload(convw_cols[:], (P, NJ * D_CONV), "convw_sb")
            lng_sb = cload(lng_cols[:], (P, KD), "lng_sb")
            lnb_sb = cload(lnb_cols[:], (P, KD), "lnb_sb")
            b1_sb = cload(b1_cols[:], (P, NJ), "b1_sb")
            b2_sb = cload(b2_cols[:], (P, 2), "b2_sb")
            ident_sb = cload(ident_b[:], (P, P), "ident_sb", bf16)
            dirmask_sb = cload(dirmask[:], (P, TCH), "dirmask_sb", u8)
            onescol_sb = cload(onescol[:], (P, 1), "onescol_sb", bf16)
            onesrow_sb = cload(onesrow[:], (1, P), "onesrow_sb", bf16)

            dbl_in_d = dram.tile([P, L], bf16, name="dbl_in_d")
            dbl_out_d = dram.tile([P, L], bf16, name="dbl_out_d")
            bcB_d = dram.tile([D_STATE, L], bf16, name="bcB_d")
            bcC_d = dram.tile([D_STATE, L], bf16, name="bcC_d")
            arm_in = [dram.tile([D_MODEL, TCH], bf16, name=f"arm_in{h}") for h in range(2)]
            arm_out = [dram.tile([D_MODEL, TCH], bf16, name=f"arm_out{h}") for h in range(2)]
            ar2_in = [dram.tile([D_MODEL, TCH], bf16, name=f"ar2_in{h}") for h in range(2)]
            rs2_out = [dram.tile([D_MODEL // 4, TCH], bf16, name=f"rs2_out{h}") for h in range(2)]

            def mm_accum(ps, lw_list, rhs_of_k, n_k):
                for k in range(n_k):
                    nc.tensor.matmul(ps[:], lw_list[k][:], rhs_of_k(k),
                                     start=(k == 0), stop=(k == n_k - 1))

            # persistent across the scan
            per_pool = stk.enter_context(tc.tile_pool(name="per_pool", bufs=1))
            sz = [per_pool.tile([P, L], bf16, tag=f"sz{j}", name=f"sz{j}")
                  for j in range(NJ)]
            wvs = [per_pool.tile([P, L], bf16, tag=f"wv{j}", name=f"wv{j}")
                   for j in range(NJ)]
            g0s = [per_pool.tile([P, L], bf16, tag=f"g0{j}", name=f"g0{j}")
                   for j in range(NJ)]
            deltas = [per_pool.tile([P, L], bf16, tag=f"delta{j}", name=f"delta{j}")
                      for j in range(NJ)]
            ygs = [per_pool.tile([P, L], bf16, tag=f"yg{j}", name=f"yg{j}")
                   for j in range(NJ)]

            # ---------------- P1..P4: produce xc, sz, delta, wv, g0 ----------------
            with tc.tile_pool(name="xc_pool", bufs=1) as xc_pool, \
                 tc.tile_pool(name="xt_pool", bufs=1) as xt_pool, \
                 tc.tile_pool(name="p1t", bufs=1) as p1t, \
                 tc.tile_pool(name="psumA", bufs=4, space="PSUM") as psumA:
                xcs = [xc_pool.tile([P, L], bf16, tag=f"xcs{j}", name=f"xcs{j}")
                       for j in range(NJ)]
                xts = []
                for k in range(KD):
                    xt_k = xt_pool.tile([P, L], bf16, tag=f"xt{k}", name=f"xt{k}")
                    nc.sync.dma_start(xt_k[:], xT[k * P:(k + 1) * P, :])
                    xts.append(xt_k)

                def in_proj_block(fb):
                    lws = []
                    for k in range(KD):
                        lw = p1t.tile([P, P], bf16, tag=f"lw{k}",
                                      name=f"lw{fb}_{k}", bufs=2)
                        nc.sync.dma_start(lw[:], w_in_t[fb, k])
                        lws.append(lw)
                    pss = []
                    for t in range(NT):
                        ps = psumA.tile([P, TCH], f32, tag="ps", name=f"inp{fb}_{t}")
                        mm_accum(ps, lws,
                                 lambda k: xts[k][:, t * TCH:(t + 1) * TCH], KD)
                        pss.append(ps)
                    return pss

                # P1: xc half + conv + silu
                for j in range(NJ):
                    xcp = p1t.tile([P, L + D_CONV - 1], bf16, tag="xcp",
                                   name=f"xcp{j}", bufs=2)
                    nc.vector.memset(xcp[:, 0:D_CONV - 1], 0.0)
                    for t, ps in enumerate(in_proj_block(j)):
                        nc.scalar.copy(
                            xcp[:, D_CONV - 1 + t * TCH:D_CONV - 1 + (t + 1) * TCH],
                            ps[:])
                    cacc = p1t.tile([P, L], bf16, tag="cacc", name=f"cacc{j}", bufs=2)
                    nc.vector.tensor_scalar_mul(
                        cacc[:], xcp[:, 0:L],
                        convw_sb[:, j * D_CONV:j * D_CONV + 1])
                    for i in range(1, D_CONV):
                        nc.vector.scalar_tensor_tensor(
                            cacc[:], xcp[:, i:i + L],
                            convw_sb[:, j * D_CONV + i:j * D_CONV + i + 1],
                            cacc[:], Alu.mult, Alu.add)
                    nc.scalar.activation(xcs[j][:], cacc[:], Act.Silu,
                                         bias=convb_sb[:, j:j + 1])

                # P2: x_proj partial over own xc half -> pairwise AllReduce
                dblp = p1t.tile([P, L], bf16, tag="dblp", name="dblp", bufs=1)
                for t in range(NT):
                    ps = psumA.tile([P, TCH], f32, tag="ps", name=f"xproj{t}")
                    for k in range(NJ):
                        lw = p1t.tile([P, P], bf16, tag="xpw", name=f"xpw{t}_{k}",
                                      bufs=2)
                        nc.sync.dma_start(lw[:], xpw_t[k])
                        nc.tensor.matmul(ps[:], lw[:],
                                         xcs[k][:, t * TCH:(t + 1) * TCH],
                                         start=(k == 0), stop=(k == NJ - 1))
                    nc.scalar.copy(dblp[:, t * TCH:(t + 1) * TCH], ps[:])
                nc.sync.dma_start(dbl_in_d[:], dblp[:])
                nc.gpsimd.collective_compute(
                    "AllReduce", Alu.add, replica_groups=_PAIRS,
                    ins=[dbl_in_d[:]], outs=[dbl_out_d[:]])

                # P3 (overlaps the pair AllReduce): z half in_proj + silu, g0
                for j in range(NJ):
                    for t, ps in enumerate(in_proj_block(NJ + j)):
                        nc.scalar.activation(sz[j][:, t * TCH:(t + 1) * TCH],
                                             ps[:], Act.Silu)
                for j in range(NJ):
                    tg = p1t.tile([P, L], bf16, tag="tg", name=f"tg{j}", bufs=2)
                    nc.vector.tensor_scalar_mul(tg[:], xcs[j][:], D_sb[:, j:j + 1])
                    nc.vector.tensor_tensor(g0s[j][:], tg[:], sz[j][:], Alu.mult)

                # P4: dt_proj + softplus -> delta; wv; broadcast B/C rows
                dbl_sb = p1t.tile([P, L], bf16, tag="dbl_sb", name="dbl_sb", bufs=1)
                nc.sync.dma_start(dbl_sb[:], dbl_out_d[:])
                for j in range(NJ):
                    lw = p1t.tile([DT_RANK, P], bf16, tag="dtw", name=f"dtw{j}",
                                  bufs=2)
                    nc.sync.dma_start(lw[:], dtw_t[j])
                    for t in range(NT):
                        ps = psumA.tile([P, TCH], f32, tag="ps", name=f"dtp{j}_{t}")
                        nc.tensor.matmul(ps[:], lw[:],
                                         dbl_sb[0:DT_RANK, t * TCH:(t + 1) * TCH],
                                         start=True, stop=True)
                        spt = p1t.tile([P, TCH], bf16, tag="spt",
                                       name=f"spt{j}_{t}", bufs=2)
                        nc.scalar.activation(spt[:], ps[:], Act.Exp,
                                             bias=dtb_sb[:, j:j + 1])
                        nc.scalar.activation(
                            deltas[j][:, t * TCH:(t + 1) * TCH], spt[:],
                            Act.Ln, bias=1.0)
                    nc.vector.tensor_tensor(wvs[j][:], deltas[j][:], xcs[j][:],
                                            Alu.mult)
                bcB_sb = p1t.tile([D_STATE, L], bf16, tag="bcB_sb", name="bcB_sb",
                                  bufs=1)
                nc.scalar.copy(bcB_sb[:], dbl_sb[DT_RANK:DT_RANK + D_STATE, :])
                nc.sync.dma_start(bcB_d[:], bcB_sb[:])
                bcC_sb = p1t.tile([D_STATE, L], bf16, tag="bcC_sb", name="bcC_sb",
                                  bufs=1)
                nc.scalar.copy(bcC_sb[:], dbl_sb[96:96 + D_STATE, :])
                nc.sync.dma_start(bcC_d[:], bcC_sb[:])

            # tail weights (outw/w1/w2) preloaded ONCE into SBUF; DMAs trickle
            # through the scan phase where the SP queue is idle.
            wq = stk.enter_context(tc.tile_pool(name="wq", bufs=1))

            def _wtiles(src, n_m, n_k, pfx):
                rows, pend = [], []
                for m in range(n_m):
                    row = []
                    for k in range(n_k):
                        t = wq.tile([P, P], bf16, tag=f"{pfx}{m}_{k}",
                                    name=f"{pfx}{m}_{k}")
                        pend.append((t, src[k, m]))
                        row.append(t)
                    rows.append(row)
                return rows, pend

            ow_sb, pend1 = _wtiles(outw_t, NJ, NJ, "owp")
            w1_sb, pend2 = _wtiles(w1_t, NJ, KD, "w1p")
            w2_sb, pend3 = _wtiles(w2_t, KD, NJ, "w2p")
            wq_pending = pend1 + pend2 + pend3

            # ---------------- P5: scan (4 j-blocks per psum wave) ----------------
            with tc.tile_pool(name="tpool", bufs=1) as tpool, \
                 tc.tile_pool(name="pscan", bufs=1, space="PSUM") as pscan:
                for hb in range(2):
                    js = list(range(hb * 4, hb * 4 + 4))
                    yps = {j: pscan.tile([P, L], f32, tag=f"yps{j % 4}",
                                         name=f"yps{j}") for j in js}
                    for n in range(D_STATE):
                        Bbc = tpool.tile([P, L], bf16, tag="Bbc",
                                         name=f"Bbc{hb}_{n}", bufs=3)
                        nc.sync.dma_start(
                            Bbc[:],
                            bcB_d[n:n + 1, :].partition_broadcast(P).squeeze(1))
                        Cbc = tpool.tile([P, L], bf16, tag="Cbc",
                                         name=f"Cbc{hb}_{n}", bufs=3)
                        nc.sync.dma_start(
                            Cbc[:],
                            bcC_d[n:n + 1, :].partition_broadcast(P).squeeze(1))
                        beng = nc.gpsimd if n in POOL_B_NS else nc.vector
                        peng = nc.gpsimd if n in POOL_P_NS else nc.vector
                        for j in js:
                            a_t = tpool.tile([P, L], bf16, tag="a_t",
                                             name=f"a{j}_{n}", bufs=3)
                            nc.scalar.activation(
                                a_t[:], deltas[j][:], Act.Exp,
                                scale=A_sb[:, j * D_STATE + n:j * D_STATE + n + 1])
                            b_t = tpool.tile([P, L], bf16, tag="b_t",
                                             name=f"b{j}_{n}", bufs=2)
                            beng.tensor_tensor(b_t[:], wvs[j][:], Bbc[:],
                                               Alu.mult)
                            h_t = tpool.tile([P, L], bf16, tag="h_t",
                                             name=f"h{j}_{n}", bufs=2)
                            nc.vector.tensor_tensor_scan(h_t[:], a_t[:], b_t[:],
                                                          0.0, Alu.mult, Alu.add)
                            prod = tpool.tile([P, L], bf16, tag="prod",
                                              name=f"p{j}_{n}", bufs=3)
                            peng.tensor_tensor(prod[:], h_t[:], Cbc[:],
                                               Alu.mult)
                            for t in range(NT):
                                sl = slice(t * TCH, (t + 1) * TCH)
                                nc.tensor.matmul(yps[j][:, sl], ident_sb[:],
                                                 prod[:, sl],
                                                 start=(n == 0),
                                                 stop=(n == D_STATE - 1))
                        for wt, srcap in wq_pending[:6]:
                            nc.sync.dma_start(wt[:], srcap)
                        del wq_pending[:6]
                    for j in js:
                        yb = tpool.tile([P, L], bf16, tag="yb", name=f"yb{j}",
                                        bufs=2)
                        nc.scalar.copy(yb[:], yps[j][:])
                        ygt = tpool.tile([P, L], bf16, tag="ygt", name=f"ygt{j}",
                                         bufs=2)
                        nc.vector.tensor_tensor(ygt[:], yb[:], sz[j][:], Alu.mult)
                        nc.vector.tensor_tensor(ygs[j][:], ygt[:], g0s[j][:],
                                                Alu.add)

            for wt, srcap in wq_pending:
                nc.sync.dma_start(wt[:], srcap)
            wq_pending.clear()

            # ---------------- P6..P8: out_proj + AR, LN, FFN per column chunk ----
            with tc.tile_pool(name="p6t", bufs=1) as p6t, \
                 tc.tile_pool(name="psumB", bufs=4, space="PSUM") as psumB, \
                 tc.tile_pool(name="pstat", bufs=1, space="PSUM") as pstat:

                def out_proj_half(ha):
                    segs = _HSEGS[ha]
                    for m in range(NJ):
                        lws = ow_sb[m]
                        ps = psumB.tile([P, TCH], f32, tag="ps", name=f"op{ha}_{m}")
                        for ci, (c0, c1) in enumerate(segs):
                            for k in range(NJ):
                                nc.tensor.matmul(
                                    ps[:, ci * 256:(ci + 1) * 256], lws[k][:],
                                    ygs[k][:, c0:c1],
                                    start=(k == 0), stop=(k == NJ - 1))
                        msb = p6t.tile([P, TCH], bf16, tag="msb",
                                       name=f"msb{ha}_{m}", bufs=2)
                        nc.scalar.copy(msb[:], ps[:])
                        nc.vector.copy_predicated(msb[:], dirmask_sb[:],
                                                  ps[:, ::-1])
                        nc.sync.dma_start(arm_in[ha][m * P:(m + 1) * P, :], msb[:])
                    nc.gpsimd.collective_compute(
                        "AllReduce", Alu.add, replica_groups=_QUADS,
                        ins=[arm_in[ha][:]], outs=[arm_out[ha][:]])

                def ln_ffn_half(ha):
                    segs = _HSEGS[ha]
                    mos = []
                    mu_ps = pstat.tile([1, TCH], f32, tag="mu_ps",
                                       name=f"mu{ha}", bufs=1)
                    e2_ps = pstat.tile([1, TCH], f32, tag="e2_ps",
                                       name=f"e2{ha}", bufs=1)
                    for m in range(KD):
                        mo = p6t.tile([P, TCH], bf16, tag=f"mo{m}",
                                      name=f"mo{ha}_{m}", bufs=1)
                        nc.sync.dma_start(mo[:], arm_out[ha][m * P:(m + 1) * P, :])
                        mos.append(mo)
                        sq = p6t.tile([P, TCH], bf16, tag="sq", name=f"sq{ha}_{m}",
                                      bufs=2)
                        nc.scalar.activation(sq[:], mo[:], Act.Square)
                        nc.tensor.matmul(mu_ps[:], onescol_sb[:], mo[:],
                                         start=(m == 0), stop=(m == KD - 1))
                        nc.tensor.matmul(e2_ps[:], onescol_sb[:], sq[:],
                                         start=(m == 0), stop=(m == KD - 1))
                    m2 = p6t.tile([1, TCH], f32, tag="m2", name=f"m2{ha}", bufs=1)
                    nc.scalar.activation(m2[:], mu_ps[:], Act.Square)
                    var_t = p6t.tile([1, TCH], f32, tag="var_t", name=f"var{ha}",
                                     bufs=1)
                    nc.vector.tensor_tensor(var_t[:], e2_ps[:], m2[:],
                                            Alu.subtract)
                    eps_sb = p6t.tile([1, 1], f32, tag="eps_sb", name=f"eps{ha}",
                                      bufs=1)
                    nc.vector.memset(eps_sb[:], 1e-5)
                    std_t = p6t.tile([1, TCH], f32, tag="std_t", name=f"std{ha}",
                                     bufs=1)
                    nc.scalar.activation(std_t[:], var_t[:], Act.Sqrt,
                                         bias=eps_sb[:])
                    rstd_b = p6t.tile([1, TCH], bf16, tag="rstd_b",
                                      name=f"rstd{ha}", bufs=1)
                    with nc.allow_low_precision(reason="bf16 rstd for broadcast"):
                        nc.vector.reciprocal(rstd_b[:], std_t[:])
                    mean_b = p6t.tile([1, TCH], bf16, tag="mean_b",
                                      name=f"mean{ha}", bufs=1)
                    nc.scalar.copy(mean_b[:], mu_ps[:])
                    mean_ps = pstat.tile([P, TCH], f32, tag="mean_ps",
                                         name=f"meanbc{ha}", bufs=1)
                    nc.tensor.matmul(mean_ps[:], onesrow_sb[:], mean_b[:],
                                     start=True, stop=True)
                    rstd_ps = pstat.tile([P, TCH], f32, tag="rstd_ps",
                                         name=f"rstdbc{ha}", bufs=1)
                    nc.tensor.matmul(rstd_ps[:], onesrow_sb[:], rstd_b[:],
                                     start=True, stop=True)
                    mean_bc = p6t.tile([P, TCH], bf16, tag="mean_bc",
                                       name=f"meanbcs{ha}", bufs=1)
                    nc.scalar.copy(mean_bc[:], mean_ps[:])
                    rstd_bc = p6t.tile([P, TCH], bf16, tag="rstd_bc",
                                       name=f"rstdbcs{ha}", bufs=1)
                    nc.scalar.copy(rstd_bc[:], rstd_ps[:])

                    xns = []
                    for m in range(KD):
                        t1 = p6t.tile([P, TCH], bf16, tag="lnt", name=f"lnt{ha}_{m}",
                                      bufs=2)
                        nc.vector.tensor_tensor(t1[:], mos[m][:], mean_bc[:],
                                                Alu.subtract)
                        nc.vector.tensor_tensor(t1[:], t1[:], rstd_bc[:], Alu.mult)
                        xn = p6t.tile([P, TCH], bf16, tag=f"xn{m}",
                                      name=f"xn{ha}_{m}", bufs=1)
                        nc.vector.tensor_scalar(xn[:], t1[:], lng_sb[:, m:m + 1],
                                                lnb_sb[:, m:m + 1], Alu.mult,
                                                Alu.add)
                        xns.append(xn)

                    ffhs = []
                    for m in range(NJ):
                        ps = psumB.tile([P, TCH], f32, tag="ps", name=f"f1{ha}_{m}")
                        mm_accum(ps, w1_sb[m], lambda k: xns[k][:], KD)
                        ffh = p6t.tile([P, TCH], bf16, tag=f"ffh{m}",
                                       name=f"ffh{ha}_{m}", bufs=1)
                        nc.scalar.activation(ffh[:], ps[:], Act.Gelu,
                                             bias=b1_sb[:, m:m + 1])
                        ffhs.append(ffh)
                    for m in range(KD):
                        ps = psumB.tile([P, TCH], f32, tag="ps", name=f"f2{ha}_{m}")
                        mm_accum(ps, w2_sb[m], lambda k: ffhs[k][:], NJ)
                        f2 = p6t.tile([P, TCH], bf16, tag="f2", name=f"f2{ha}_{m}",
                                      bufs=2)
                        nc.scalar.copy(f2[:], ps[:])
                        nc.sync.dma_start(ar2_in[ha][m * P:(m + 1) * P, :], f2[:])
                    nc.gpsimd.collective_compute(
                        "ReduceScatter", Alu.add, replica_groups=_QUADS,
                        ins=[ar2_in[ha][:]], outs=[rs2_out[ha][:]])

                def store_half(ha):
                    segs = _HSEGS[ha]
                    for q in range(2):
                        fin = p6t.tile([P, TCH], bf16, tag="fin", name=f"fin{ha}_{q}",
                                       bufs=2)
                        nc.sync.dma_start(fin[:], rs2_out[ha][q * P:(q + 1) * P, :])
                        fob = p6t.tile([P, TCH], f32, tag="fob", name=f"fob{ha}_{q}",
                                       bufs=2)
                        nc.vector.tensor_scalar_add(fob[:], fin[:],
                                                    b2_sb[:, q:q + 1])
                        for ci, (c0, c1) in enumerate(segs):
                            nc.sync.dma_start(
                                out_m[q * P:(q + 1) * P, c0:c1],
                                fob[:, ci * 256:(ci + 1) * 256])

                out_proj_half(0)
                out_proj_half(1)
                ln_ffn_half(0)
                ln_ffn_half(1)
                store_half(0)
                store_half(1)

    nc.compile()
    return nc


def _prep_inputs(inputs):
    """Per-core input dicts. Core c: sequence s=c//2 (s>=2 => time-flipped x),
    d_inner half = c%2. The own half of d_inner is permuted FIRST in every
    d_inner-ordered tensor, so the device kernel is identical on all cores."""
    x = np.asarray(inputs["x"], dtype=np.float32)
    in_proj_w = np.asarray(inputs["in_proj_w"], dtype=np.float32)
    conv_w = np.asarray(inputs["conv_w"], dtype=np.float32)
    conv_b = np.asarray(inputs["conv_b"], dtype=np.float32)
    x_proj_w = np.asarray(inputs["x_proj_w"], dtype=np.float32)
    dt_proj_w = np.asarray(inputs["dt_proj_w"], dtype=np.float32)
    dt_proj_b = np.asarray(inputs["dt_proj_b"], dtype=np.float32)
    A = -np.exp(np.asarray(inputs["A_log"], dtype=np.float32))
    Dp = np.asarray(inputs["D"], dtype=np.float32)
    out_proj_w = np.asarray(inputs["out_proj_w"], dtype=np.float32)
    ln_g = np.asarray(inputs["ln_g"], dtype=np.float32)
    ln_b = np.asarray(inputs["ln_b"], dtype=np.float32)
    ff_w1 = np.asarray(inputs["ff_w1"], dtype=np.float32)
    ff_b1 = np.asarray(inputs["ff_b1"], dtype=np.float32)
    ff_w2 = np.asarray(inputs["ff_w2"], dtype=np.float32)
    ff_b2 = np.asarray(inputs["ff_b2"], dtype=np.float32)

    def cols(v):  # (N,) -> (P, N//P) per-partition column layout
        return np.ascontiguousarray(v.reshape(-1, P).T)

    def tile_w(w, KP, MP):  # (K, M) -> (K//KP, M//MP, KP, MP) bf16
        K, M = w.shape
        return np.ascontiguousarray(
            w.reshape(K // KP, KP, M // MP, MP).transpose(0, 2, 1, 3)
        ).astype(BF16)

    in_maps = []
    for c in range(8):
        s, half = c // 2, c % 2
        xb = x[s] if s < 2 else x[s - 2][::-1]
        perm = np.arange(D_INNER).reshape(2, HALF)
        own = np.concatenate([perm[half], perm[1 - half]])[:HALF]

        wxc = in_proj_w[:, own]                               # (1024, 1024)
        wz = in_proj_w[:, D_INNER + own]                      # (1024, 1024)
        w_in = np.concatenate([wxc, wz], axis=1)              # (1024, 2048)
        w_in_t = np.ascontiguousarray(
            tile_w(w_in, P, P).transpose(1, 0, 2, 3))         # (16 fb, 8 k, P, P)

        cw = conv_w[own]  # (1024, 4) -> (P, 8*4): col j*4+i = w[jP+p, i]
        convw_cols = np.ascontiguousarray(
            cw.reshape(NJ, P, D_CONV).transpose(1, 0, 2).reshape(P, NJ * D_CONV))

        g = (c & 1) + 2 * (c >> 2)
        hsl = slice(g * FF_SLICE, (g + 1) * FF_SLICE)

        in_maps.append({
            "xT": np.ascontiguousarray(xb.T).astype(BF16),
            "w_in_t": w_in_t,
            "convw_cols": convw_cols,
            "convb_cols": cols(conv_b[own]),
            "xpw_t": np.ascontiguousarray(
                np.concatenate([
                    x_proj_w[own][:, :DT_RANK + D_STATE],
                    np.zeros((HALF, D_STATE), np.float32),
                    x_proj_w[own][:, DT_RANK + D_STATE:],
                    np.zeros((HALF, D_STATE), np.float32),
                ], axis=1).reshape(NJ, P, P)).astype(BF16),
            "dtw_t": np.ascontiguousarray(
                dt_proj_w[:, own].reshape(DT_RANK, NJ, P).transpose(1, 0, 2)
            ).astype(BF16),
            "dtb_cols": cols(dt_proj_b[own]),
            "A_cols": np.ascontiguousarray(
                A[own].reshape(NJ, P, D_STATE).transpose(1, 0, 2).reshape(
                    P, NJ * D_STATE)),
            "D_colsT": cols(Dp[own]),
            "outw_t": tile_w(out_proj_w[own], P, P),
            "lng_cols": cols(ln_g),
            "lnb_cols": cols(ln_b),
            "w1_t": tile_w(ff_w1[:, hsl], P, P),
            "b1_cols": cols(ff_b1[hsl]),
            "w2_t": tile_w(ff_w2[hsl], P, P),
            "b2_cols": cols(ff_b2[g * 256:(g + 1) * 256]),
            "ident_b": np.eye(P, dtype=np.float32).astype(BF16),
            "dirmask": np.full((P, TCH), 1 if s >= 2 else 0, np.uint8),
            "onescol": np.full((P, 1), 1.0 / 1024.0, np.float32).astype(BF16),
            "onesrow": np.ones((1, P), np.float32).astype(BF16),
        })
    return in_maps


_NC_CACHE = {}


def _get_nc():
    if "nc" not in _NC_CACHE:
        _NC_CACHE["nc"] = _build_nc()
    return _NC_CACHE["nc"]


def run(inputs, trace=False):
    _install_ntff_hook_shim()
    from concourse import bass_utils
    nc = _get_nc()
    in_maps = _prep_inputs(inputs)
    res = bass_utils.run_bass_kernel_spmd(nc, in_maps, core_ids=list(range(8)),
                                          trace=trace)
    # each core holds the dm-quarter (rows g*256..) of its group's output
    full = np.zeros((2, D_MODEL, L), np.float32)
    for c in range(8):
        b = 0 if c in (0, 1, 4, 5) else 1
        g = (c & 1) + 2 * (c >> 2)
        full[b, g * 256:(g + 1) * 256, :] = res.results[c]["out_m"]
    out = np.ascontiguousarray(full.transpose(0, 2, 1))
    return out, res


def kernel(**inputs):
    out, _ = run(inputs, trace=False)
    return out
